# revision 3
# baseline (speedup 1.0000x reference)
"""Trainium2 Bass kernel for nn_DiscriminationLoss (segment_reduce).

Strategy (8 NeuronCores, pixel-sharded):
  - Each core gets 1/8 of the 4M pixels: pred slice [8, 524288] f32 and
    labels slice [524288] i32.
  - Pixels are tiled [128 partitions x F free]. For each free column t
    (a "block" of 128 pixels), a one-hot matrix oh[p, j] = (labels[p,t]
    == j+1), j in 0..31 is built on DVE (label 0 = background dropped).
  - One-hot generation uses per-label tensor_scalar(is_equal) ops: a
    single-source op with dense step-1 16-bit APs engages the DVE
    4x_2p perf mode (tensor_tensor caps at 2x_1p). Measured per-op
    engine cost ~ (58 + FD/4) cycles @0.96 GHz, so chunks are large
    (up to 1024 blocks) to amortize the fixed 58 cycles; ~44us total.
  - The pixel<->(partition, column) mapping is chunk-local:
    pixel = 128*coff + p*fcc + f for chunk [coff, coff+fcc). Labels
    are cast-DMA'd (int32->bf16, SWDGE) into per-chunk resident tiles
    at kernel start; pred group DMAs use the same chunk mapping so
    label/pred columns line up.
  - pred is scaled by 2^14 and cast to bf16 on ScalarE (the scale
    rides the activation's free affine; a second activation with
    scale=0, bias=1 writes the ones column used for counts).
  - The PE contracts QB=8 blocks per matmul (block-diagonal batching):
      psum[72, 256] += st[128, 72]^T @ oh[128, 256]
    where st packs 8 blocks' [8 bf16 channels | ones] side by side and
    oh packs their one-hots; only the 8 diagonal [9, 32] sub-blocks of
    the product are meaningful and the host extracts them. PSUM
    accumulates over all 512 matmuls per core. Mid-kernel matmul gap
    measures ~109 ns = the 256-col @2.4 GHz floor with LDWEIGHTS
    hidden. A short warmup burst of matmuls first trips the PE HAM
    clock gate to 2.4 GHz.
  - Chunk sizes ramp [128, 896, 1024, 1024, 1024] so the first one-hot
    chunk and first pred cast land early and the PE pipeline fills
    with minimal head latency.
  - Each core emits [128, 256] (PSUM readout + warmup dump row). Host
    sums partials over cores (the "psum" step of the sharding hint)
    and evaluates the tiny O(K^2) pairwise tail in f64.
"""

import sys
import functools

sys.path.insert(0, "/opt/trn_rl_repo")

import numpy as np

C = 8
K = 32
NCORES = 8
H = W = 2048
PTOT = H * W
PCORE = PTOT // NCORES  # 524288
SIGMA_DIS = 3.0
PRED_SCALE = float(2.0**14)

QB = 8     # pixel-blocks batched per matmul (block-diagonal trick)
WARM_MMS = 20  # PE warmup matmuls (trip the HAM clock gate to 2.4 GHz)

# chunk sizes (one-hot + label granularity); each chunk is split into
# DMA/cast groups of <= 512 columns
CHUNKS = [128, 896, 1024, 1024, 1024]
GSPLIT = {128: [128], 896: [384, 512], 1024: [512, 512]}


def build_nc(pcore=PCORE, qb=QB, warm=WARM_MMS):
    import concourse.bacc as bacc
    import concourse.tile as tile
    import concourse.mybir as mybir
    from contextlib import ExitStack

    ftot = pcore // 128
    assert pcore % 128 == 0
    assert sum(CHUNKS) == ftot
    f32 = mybir.dt.float32
    bf16 = mybir.dt.bfloat16
    i32 = mybir.dt.int32

    nch = C + 1
    ones_col = C
    fgmax = 512
    fcmax = max(CHUNKS)

    nc = bacc.Bacc(
        "TRN2", target_bir_lowering=False, debug=False, num_devices=NCORES
    )
    pred_ext = nc.dram_tensor("pred", [C, pcore], f32, kind="ExternalInput")
    lab_ext = nc.dram_tensor("labels", [pcore], i32, kind="ExternalInput")
    # rows 0..nch*qb-1: results; row 96: warmup dump (keeps warm MMs live)
    out_ext = nc.dram_tensor("out_s", [128, K * qb], f32, kind="ExternalOutput")

    with tile.TileContext(nc) as tc, ExitStack() as ctx:
        const_pool = ctx.enter_context(tc.tile_pool(name="const", bufs=1))
        lab_pool = ctx.enter_context(tc.tile_pool(name="lab", bufs=1))
        slab32_pool = ctx.enter_context(tc.tile_pool(name="slab32", bufs=2))
        slabh_pool = ctx.enter_context(tc.tile_pool(name="slabh", bufs=3))
        oh_pool = ctx.enter_context(tc.tile_pool(name="oh", bufs=2))
        psum_pool = ctx.enter_context(tc.tile_pool(name="psum", bufs=1, space="PSUM"))
        out_pool = ctx.enter_context(tc.tile_pool(name="outp", bufs=1))

        # warm tile: memset early (no input deps) — feeds PE warmup and
        # the ones-column activations
        warm_t = const_pool.tile([128, 256], bf16)
        nc.gpsimd.memset(warm_t[:], 1.0)

        # all labels resident up front, one tile per chunk with the
        # chunk-local pixel mapping; SWDGE cast-DMA int32 -> bf16
        lab_ts = []
        coff = 0
        for ci, fcc in enumerate(CHUNKS):
            lt = lab_pool.tile([128, fcc], bf16, tag=f"lab{ci}")
            nc.gpsimd.dma_start(
                lt[:],
                lab_ext[128 * coff : 128 * (coff + fcc)].rearrange(
                    "(p f) -> p f", p=128
                ),
            )
            lab_ts.append(lt)
            coff += fcc

        psum_full = psum_pool.tile([128, K * qb], f32)
        psum_t = psum_full[: nch * qb, :]

        # PE warmup: dense matmuls so the HAM clock gate opens before
        # the real matmul stream arrives.
        warm_ps = psum_pool.tile([128, 256], f32)
        if warm:
            for w in range(warm):
                nc.tensor.matmul(
                    warm_ps[:],
                    warm_t[:, :128],
                    warm_t[:, :256],
                    start=(w == 0),
                    stop=(w == warm - 1),
                )

        nblocks = ftot
        blk = 0
        coff = 0
        for ci, fcc in enumerate(CHUNKS):
            # chunk-local view of pred: [p, c, f] with pixel =
            # 128*coff + p*fcc + f
            pred_chunk = pred_ext[:, 128 * coff : 128 * (coff + fcc)].rearrange(
                "c (p f) -> p c f", p=128
            )
            # per-group DMA + cast
            slabhs = []
            goff = 0
            for fgg in GSPLIT[fcc]:
                slab32 = slab32_pool.tile([128, C * fgmax], f32, tag="slab32")
                s32 = slab32[:, : C * fgg]
                nc.sync.dma_start(
                    s32.rearrange("p (c f) -> p c f", c=C),
                    pred_chunk[:, :, goff : goff + fgg],
                )
                # slabh layout: [p, (tg, c, b)] — each tg-group's
                # stationary [128, nch*qb] is a contiguous slice.
                slabh = slabh_pool.tile([128, nch * fgmax], bf16, tag="slabh")
                slabh_r = slabh[:, : nch * fgg].rearrange(
                    "p (tg c b) -> p tg c b", c=nch, b=qb
                )
                slab32_r = s32.rearrange("p (c tg b) -> p tg c b", c=C, b=qb)
                # scaled bf16 cast on ScalarE: out = Copy(in * 2^14)
                nc.scalar.activation(
                    slabh_r[:, :, :C, :],
                    slab32_r,
                    mybir.ActivationFunctionType.Copy,
                    scale=PRED_SCALE,
                )
                # ones column via ACT: Copy(0*x + 1) = 1.0. Input is
                # warm_t (always ready) so this op has no false
                # dependency on the pred DMA and can run early.
                nc.scalar.activation(
                    slabh_r[:, :, ones_col, :],
                    warm_t[:, :1].unsqueeze(2).broadcast_to([128, fgg // qb, qb]),
                    mybir.ActivationFunctionType.Copy,
                    bias=1.0,
                    scale=0.0,
                )
                slabhs.append((goff, fgg, slabh))
                goff += fgg

            # one-hot chunk: per-label tensor_scalar(is_equal) at DVE 4x.
            # oh layout: [p, (tg, j, b)] — each tg-group's moving
            # operand [128, K*qb] is a contiguous slice.
            oh = oh_pool.tile([128, K * fcmax], bf16, tag="oh")
            oh_r = oh[:, : K * fcc].rearrange(
                "p (tg j b) -> p tg j b", j=K, b=qb
            )  # [128, fcc/qb, K, qb]
            lab_in = lab_ts[ci][:].rearrange("p (tg b) -> p tg b", b=qb)
            for j in range(K):
                nc.vector.tensor_scalar(
                    oh_r[:, :, j, :],
                    lab_in,
                    float(j + 1),
                    None,
                    mybir.AluOpType.is_equal,
                )
            # matmuls: stationary from the owning group's slabh
            for goff, fgg, slabh in slabhs:
                for tgl in range(fgg // qb):
                    tg = goff // qb + tgl  # chunk-local tg
                    nc.tensor.matmul(
                        psum_t[:],
                        slabh[:, tgl * nch * qb : (tgl + 1) * nch * qb],
                        oh[:, tg * K * qb : (tg + 1) * K * qb],
                        start=(blk == 0),
                        stop=(blk == nblocks - qb),
                    )
                    blk += qb
            coff += fcc

        outt = out_pool.tile([128, K * qb], f32)
        nc.vector.memset(outt[:], 0.0)
        nc.vector.tensor_copy(outt[: nch * qb, :], psum_t[:])
        if warm:
            nc.vector.tensor_copy(outt[96:97, :], warm_ps[96:97, : K * qb])
        nc.sync.dma_start(out_ext[:], outt[:])
    nc.compile()
    return nc


@functools.lru_cache(maxsize=1)
def _get_program():
    return build_nc()


def make_in_maps(pred_flat, labels_flat):
    in_maps = []
    for i in range(NCORES):
        sl = slice(i * PCORE, (i + 1) * PCORE)
        in_maps.append(
            {
                "pred": np.ascontiguousarray(pred_flat[:, sl]),
                "labels": np.ascontiguousarray(labels_flat[sl]),
            }
        )
    return in_maps


def finish_host(parts, num_kernel, qb=QB):
    """parts: per-core [128, K*qb] partials. Tiny O(K^2) tail in f64."""
    nch = C + 1
    total = np.sum([p.astype(np.float64) for p in parts], axis=0)
    r = total[: nch * qb, :].reshape(nch, qb, K, qb)
    total = r[:, np.arange(qb), :, np.arange(qb)].sum(axis=0)  # [nch, K]
    S = total[:C, :] / PRED_SCALE  # [8, 32]
    N = total[C, :]  # [32]
    A = N * np.sum(S * S, axis=0)  # [32]
    kk = int(num_kernel)
    A = A[:kk]
    pair = A[:, None] + A[None, :]
    Dm = np.maximum(SIGMA_DIS - np.sqrt(pair), 0.0)
    term = np.log(Dm * Dm + 1.0)
    L = float(np.sum(np.triu(term, k=1)))
    L *= (kk - 1) / kk
    return np.float32(L)


_last_results = None


def kernel(pred_similarities, regions_mask, kernel_labels, num_kernel, **kw):
    global _last_results
    from concourse.bass_utils import run_bass_kernel_spmd

    pred_flat = np.asarray(pred_similarities, dtype=np.float32).reshape(C, PTOT)
    labels_flat = np.asarray(kernel_labels, dtype=np.int32).reshape(PTOT)

    nc = _get_program()
    in_maps = make_in_maps(pred_flat, labels_flat)
    res = run_bass_kernel_spmd(nc, in_maps, list(range(NCORES)))
    _last_results = res
    parts = [res.results[i]["out_s"] for i in range(NCORES)]
    return finish_host(parts, num_kernel)


# revision 6
# speedup vs baseline: 1.0823x; 1.0823x over previous
"""Trainium2 Bass kernel for nn_DiscriminationLoss (segment_reduce).

Strategy (8 NeuronCores, pixel-sharded):
  - Each core gets 1/8 of the 4M pixels: pred slice [8, 524288] f32 and
    labels slice [524288] i32.
  - Pixels are tiled [128 partitions x F free]. For each free column t
    (a "block" of 128 pixels), a one-hot matrix oh[p, j] = (labels[p,t]
    == j+1), j in 0..31 is built on DVE (label 0 = background dropped).
  - One-hot generation uses per-label tensor_scalar(is_equal) ops: a
    single-source op with dense step-1 16-bit APs engages the DVE
    4x_2p perf mode (tensor_tensor caps at 2x_1p). Measured per-op
    engine cost ~ (58 + FD/4) cycles @0.96 GHz, so chunks are large
    (~900 blocks) to amortize the fixed cost; ~44us total and this is
    the body's critical path together with the ~47us HBM stream.
  - The pixel<->(partition, column) mapping is chunk-local:
    pixel = 128*coff + p*fcc + f for chunk [coff, coff+fcc). Labels
    are cast-DMA'd (int32->bf16, SWDGE) into per-chunk resident tiles
    at kernel start; pred group DMAs use the same chunk mapping.
  - pred is scaled by 2^14 and cast to bf16 on ScalarE (the scale
    rides the activation's free affine; a second activation with
    scale=0, bias=1 writes the ones column used for counts).
  - PE uses 2-way COLUMN TILING: per 12 blocks, two matmuls run
    concurrently in disjoint 64-column strips of the 128x128 array
    (tile_position (0,0) / (0,64), inferred from the PSUM slice base
    partition):
      psumA[54, 192] += stA[128, 54]^T @ ohA[128, 192]   (cols 0..63)
      psumB[54, 192] += stB[128, 54]^T @ ohB[128, 192]   (cols 64..127)
    st packs 6 blocks' [8 bf16 channels | ones]; oh packs 6 blocks'
    one-hots; only the 6 diagonal [9, 32] sub-blocks per matmul are
    meaningful (host extracts them). Col-tiling roughly halves PE
    streaming time vs a single 72-col stream, so the PE is no longer
    the pacer. The last 496 blocks use qb=4 into a separate PSUM
    column range (its accumulation group runs strictly after the qb=6
    groups, so its has_written bank-clear cannot corrupt them).
  - A short warmup burst of matmuls trips the PE HAM clock gate to
    2.4 GHz before the real stream arrives.
  - Each core emits [128, 512] (PSUM readout + warmup dump row). Host
    sums partials over cores (the "psum" step of the sharding hint)
    and evaluates the tiny O(K^2) pairwise tail in f64.
"""

import sys
import functools

sys.path.insert(0, "/opt/trn_rl_repo")

import numpy as np

C = 8
K = 32
NCORES = 8
H = W = 2048
PTOT = H * W
PCORE = PTOT // NCORES  # 524288
SIGMA_DIS = 3.0
PRED_SCALE = float(2.0**14)

WARM_MMS = 20  # PE warmup matmuls (trip the HAM clock gate to 2.4 GHz)

# (chunk_cols, qb) — chunk = one-hot/label granularity. qb=6 chunks use
# 2-way col tiling (2*54=108 array cols); the final qb=4 chunk mops up
# the remainder (2*36 cols) into a separate PSUM column range.
CHUNKS = [(900, 6), (900, 6), (900, 6), (900, 6), (496, 4)]
GSPLIT = {900: [450, 450], 496: [496]}


def build_nc(pcore=PCORE, warm=WARM_MMS):
    import concourse.bacc as bacc
    import concourse.tile as tile
    import concourse.mybir as mybir
    from contextlib import ExitStack

    ftot = pcore // 128
    assert sum(c for c, _ in CHUNKS) == ftot
    f32 = mybir.dt.float32
    bf16 = mybir.dt.bfloat16
    i32 = mybir.dt.int32

    nch = C + 1
    ones_col = C
    fgmax = 512
    fcmax = max(c for c, _ in CHUNKS)

    nc = bacc.Bacc(
        "TRN2", target_bir_lowering=False, debug=False, num_devices=NCORES
    )
    pred_ext = nc.dram_tensor("pred", [C, pcore], f32, kind="ExternalInput")
    lab_ext = nc.dram_tensor("labels", [pcore], i32, kind="ExternalInput")
    out_ext = nc.dram_tensor("out_s", [128, 512], f32, kind="ExternalOutput")

    with tile.TileContext(nc) as tc, ExitStack() as ctx:
        const_pool = ctx.enter_context(tc.tile_pool(name="const", bufs=1))
        lab_pool = ctx.enter_context(tc.tile_pool(name="lab", bufs=1))
        slab32_pool = ctx.enter_context(tc.tile_pool(name="slab32", bufs=2))
        slabh_pool = ctx.enter_context(tc.tile_pool(name="slabh", bufs=3))
        oh_pool = ctx.enter_context(tc.tile_pool(name="oh", bufs=2))
        psum_pool = ctx.enter_context(tc.tile_pool(name="psum", bufs=1, space="PSUM"))
        out_pool = ctx.enter_context(tc.tile_pool(name="outp", bufs=1))

        # warm tile: memset early (no input deps) — feeds PE warmup and
        # the ones-column activations
        warm_t = const_pool.tile([128, 256], bf16)
        nc.gpsimd.memset(warm_t[:], 1.0)

        # all labels resident up front, one tile per chunk with the
        # chunk-local pixel mapping; SWDGE cast-DMA int32 -> bf16
        lab_ts = []
        coff = 0
        for ci, (fcc, _) in enumerate(CHUNKS):
            lt = lab_pool.tile([128, fcc], bf16, tag=f"lab{ci}")
            nc.gpsimd.dma_start(
                lt[:],
                lab_ext[128 * coff : 128 * (coff + fcc)].rearrange(
                    "(p f) -> p f", p=128
                ),
            )
            lab_ts.append(lt)
            coff += fcc

        psum_full = psum_pool.tile([128, 512], f32)

        # PE warmup: dense matmuls so the HAM clock gate opens before
        # the real matmul stream arrives.
        warm_ps = psum_pool.tile([128, 256], f32)
        if warm:
            for w in range(warm):
                nc.tensor.matmul(
                    warm_ps[:],
                    warm_t[:, :128],
                    warm_t[:, :256],
                    start=(w == 0),
                    stop=(w == warm - 1),
                )

        # per-(qb, half) accumulation bookkeeping:
        #   qb=6 -> psum cols 0..191, qb=4 -> cols 192..319
        # first/last matmul of each (qb, half) group carries start/stop.
        npair = {6: 0, 4: 0}
        for fcc, qb in CHUNKS:
            npair[qb] += fcc // (2 * qb)
        seen = {6: 0, 4: 0}

        coff = 0
        for ci, (fcc, qb) in enumerate(CHUNKS):
            stw = nch * qb          # stationary cols per matmul
            mvw = K * qb            # moving cols per matmul
            colbase = 0 if qb == 6 else 192
            pred_chunk = pred_ext[:, 128 * coff : 128 * (coff + fcc)].rearrange(
                "c (p f) -> p c f", p=128
            )
            # per-group DMA + cast
            slabhs = []
            goff = 0
            for fgg in GSPLIT[fcc]:
                slab32 = slab32_pool.tile([128, C * fgmax], f32, tag="slab32")
                s32 = slab32[:, : C * fgg]
                nc.sync.dma_start(
                    s32.rearrange("p (c f) -> p c f", c=C),
                    pred_chunk[:, :, goff : goff + fgg],
                )
                # slabh layout: [p, (tg, c, b)] — each tg's stationary
                # [128, nch*qb] is a contiguous slice.
                slabh = slabh_pool.tile([128, nch * fgmax], bf16, tag="slabh")
                slabh_r = slabh[:, : nch * fgg].rearrange(
                    "p (tg c b) -> p tg c b", c=nch, b=qb
                )
                slab32_r = s32.rearrange("p (c tg b) -> p tg c b", c=C, b=qb)
                # scaled bf16 cast on ScalarE: out = Copy(in * 2^14)
                nc.scalar.activation(
                    slabh_r[:, :, :C, :],
                    slab32_r,
                    mybir.ActivationFunctionType.Copy,
                    scale=PRED_SCALE,
                )
                # ones column via ACT: Copy(0*x + 1) = 1.0; input warm_t
                # (always ready) so this op can run before the pred DMA.
                nc.scalar.activation(
                    slabh_r[:, :, ones_col, :],
                    warm_t[:, :1].unsqueeze(2).broadcast_to([128, fgg // qb, qb]),
                    mybir.ActivationFunctionType.Copy,
                    bias=1.0,
                    scale=0.0,
                )
                slabhs.append((goff, fgg, slabh))
                goff += fgg

            # one-hot chunk: per-label tensor_scalar(is_equal) at DVE 4x.
            # oh layout: [p, (tg, j, b)] — each tg's moving operand
            # [128, K*qb] is a contiguous slice.
            oh = oh_pool.tile([128, K * fcmax], bf16, tag="oh")
            oh_r = oh[:, : K * fcc].rearrange(
                "p (tg j b) -> p tg j b", j=K, b=qb
            )  # [128, fcc/qb, K, qb]
            lab_in = lab_ts[ci][:].rearrange("p (tg b) -> p tg b", b=qb)
            for j in range(K):
                nc.vector.tensor_scalar(
                    oh_r[:, :, j, :],
                    lab_in,
                    float(j + 1),
                    None,
                    mybir.AluOpType.is_equal,
                )
            # col-tiled matmul pairs: tg even -> array cols 0..63
            # (psum partitions 0..stw-1), tg odd -> cols 64..127
            # (psum partitions 64..64+stw-1)
            for goff, fgg, slabh in slabhs:
                for tgl in range(fgg // qb):
                    tg = goff // qb + tgl  # chunk-local tg
                    half = tg % 2
                    pbase = 64 * half
                    nc.tensor.matmul(
                        psum_full[pbase : pbase + stw, colbase : colbase + mvw],
                        slabh[:, tgl * stw : (tgl + 1) * stw],
                        oh[:, tg * mvw : (tg + 1) * mvw],
                        start=(seen[qb] == 0 or seen[qb] == 1),
                        stop=(seen[qb] == 2 * npair[qb] - 2
                              or seen[qb] == 2 * npair[qb] - 1),
                        skip_group_check=True,
                    )
                    seen[qb] += 1
            coff += fcc

        outt = out_pool.tile([128, 512], f32)
        nc.vector.memset(outt[:], 0.0)
        nc.vector.tensor_copy(outt[:118, :320], psum_full[:118, :320])
        if warm:
            nc.vector.tensor_copy(outt[96:97, 320:512], warm_ps[96:97, :192])
        nc.sync.dma_start(out_ext[:], outt[:])
    nc.compile()
    return nc


@functools.lru_cache(maxsize=1)
def _get_program():
    return build_nc()


def make_in_maps(pred_flat, labels_flat):
    in_maps = []
    for i in range(NCORES):
        sl = slice(i * PCORE, (i + 1) * PCORE)
        in_maps.append(
            {
                "pred": np.ascontiguousarray(pred_flat[:, sl]),
                "labels": np.ascontiguousarray(labels_flat[sl]),
            }
        )
    return in_maps


def finish_host(parts, num_kernel):
    """parts: per-core [128, 512] partials. Tiny O(K^2) tail in f64."""
    nch = C + 1
    total = np.sum([p.astype(np.float64) for p in parts], axis=0)
    acc = np.zeros((nch, K))
    for qb, colbase in ((6, 0), (4, 192)):
        for pbase in (0, 64):
            r = total[pbase : pbase + nch * qb, colbase : colbase + K * qb]
            r = r.reshape(nch, qb, K, qb)
            acc += r[:, np.arange(qb), :, np.arange(qb)].sum(axis=0)
    S = acc[:C, :] / PRED_SCALE  # [8, 32]
    N = acc[C, :]  # [32]
    A = N * np.sum(S * S, axis=0)  # [32]
    kk = int(num_kernel)
    A = A[:kk]
    pair = A[:, None] + A[None, :]
    Dm = np.maximum(SIGMA_DIS - np.sqrt(pair), 0.0)
    term = np.log(Dm * Dm + 1.0)
    L = float(np.sum(np.triu(term, k=1)))
    L *= (kk - 1) / kk
    return np.float32(L)


_last_results = None


def kernel(pred_similarities, regions_mask, kernel_labels, num_kernel, **kw):
    global _last_results
    from concourse.bass_utils import run_bass_kernel_spmd

    pred_flat = np.asarray(pred_similarities, dtype=np.float32).reshape(C, PTOT)
    labels_flat = np.asarray(kernel_labels, dtype=np.int32).reshape(PTOT)

    nc = _get_program()
    in_maps = make_in_maps(pred_flat, labels_flat)
    res = run_bass_kernel_spmd(nc, in_maps, list(range(NCORES)))
    _last_results = res
    parts = [res.results[i]["out_s"] for i in range(NCORES)]
    return finish_host(parts, num_kernel)
